# revision 48
# baseline (speedup 1.0000x reference)
"""Binary-weight 3x3 conv (sign(W)), NCHW, stride 1, pad 1, on 8 trn2 cores.

Data-parallel over batch (4 images/core), implicit GEMM in fp8 DoubleRow.

Numerics: x is split host-side into x_hi = e4m3(x) and x_lo = e4m3(x - x_hi),
row-interleaved in one padded buffer. The contraction runs hi taps (all 9)
plus a PARTIAL residual (lo) correction, packed two-per-DoubleRow-matmul.
The correction is ASYMMETRIC across the two output-channel halves: half 0
corrects 5 taps (center + 4 edges -> 14 slots -> 7 DR matmuls), half 1
corrects 3 taps (center + 2 edges -> 12 slots -> 6 DR matmuls). Total
13 DR matmuls per (img, row-block) instead of 14: ~7% fewer PE cycles.
Correcting interior taps first is variance-optimal (border taps multiply
zero padding on 1/56 of outputs, so they carry slightly less error).

Error model (verified to 0.5% against HW): e4m3 quantization gives rel L2
0.02667; correcting a tap of weight w_t (center 1.0, edge 55/56, corner
(55/56)^2, sum 8.787) scales the variance by (8.787 - sum w_corr)/8.787
per half. Host-sim of this scheme = 0.01969; HW adds ~0.0017 in quadrature
(bf16 output store) -> ~0.01976 < 2e-2.

HW constraints honored: the DR ifmap group stride (dim-1 of the custom
4-dim access pattern) must be even and not tiny -- odd strides fault the
device -- so each parity class (by kw, since WP is even) must have an even
slot count; pairs are first-half x second-half within a class (deltas
116/174 elements, all even).

Schedule: DMA completion cost is per-packet (one packet per partition,
~110ns each across 16 HW engines that serve all queues round-robin), so
ALL input DMAs ride sync's single queue in need-order: a 16-packet dummy
that attaches every HW engine to the queue (the 16th otherwise joins
~1.9us late and its slice straggles), then one <=2KB/partition gating
packet carrying img0's first rows plus the first 3 weight pairs, then the
rest in stream order. Output stores ride scalar's queue so they never
queue ahead of inputs; gpsimd (whose exit drain scales with its queue
depth) carries only the final two tiny stores. Uninitialized-scratch
warmup matmuls bridge preamble-end to gating-data-ready with no PE idle
gap, so the HAM clock ramp (needs ~3.4us of continuous PE busy to reach
2.4 GHz) completes before the real stream starts. PSUM is drained to bf16
(halving store traffic, upcast on host) on alternating Vector/Scalar
engines; the final block is computed as two 4-row PSUM groups with
independent ot tiles across engines/queues so the last drain+store chain
is as short as possible.
"""

import numpy as np

import concourse.bacc as bacc
import concourse.mybir as mybir
from concourse.ap import AP
from concourse.tile import TileContext
from concourse.bass_utils import run_bass_kernel_spmd

N_CORES = 8
IMGS = 4
C = 128
O = 256
H = WD = 56
HP = WP = 58
KH = KW = 3
RB = 8
NBLK = H // RB
P = 128
# warmup matmuls bridge from preamble-end (~7.5us) to gating-data-ready
# with NO PE idle gap, so the HAM clock ramp (needs ~3.4us of continuous
# PE busy) completes before the stream. A 16-packet dummy DMA rides ahead
# of the gate so all 16 HW DMA engines attach to the queue early (the 16th
# engine otherwise attaches ~1.9us after the doorbell and its slice of the
# gate becomes a ~1.5us straggler).
N_WARM = 10

CHUNKS = [(0, 10), (8, 18), (24, 18), (40, 18)]
BLK_CHUNK = [0, 1, 1, 2, 2, 3, 3]

F32 = mybir.dt.float32
BF16 = mybir.dt.bfloat16
FP8 = mybir.dt.float8e4
NP_FP8 = mybir.dt.np(FP8)
DR = mybir.MatmulPerfMode.DoubleRow

# Residual (lo) taps corrected per output-channel half, interior-first
# (variance-optimal) while keeping each kw-parity class even-sized for
# DoubleRow pairing.
LO_TAPS = [
    [(1, 0), (1, 2), (0, 1), (1, 1), (2, 1)],  # half 0: center + 4 edges
    [(1, 0), (1, 2), (1, 1)],                  # half 1: center + 2 edges
]


# slot = (lvl, kh, kw); offset within an image-row block = kh*2*WP + lvl*WP + kw
def _off0(slot):
    lvl, kh, kw = slot
    return kh * 2 * WP + lvl * WP + kw


def _make_pairs(lo_taps):
    slots = [(0, kh, kw) for kh in range(KH) for kw in range(KW)]
    slots += [(1, kh, kw) for kh, kw in lo_taps]
    pairs = []
    for parity in (0, 1):
        cls = sorted((s for s in slots if _off0(s) % 2 == parity), key=_off0)
        assert len(cls) % 2 == 0, (parity, cls)
        h = len(cls) // 2
        pairs += list(zip(cls[:h], cls[h:]))
    for sa, sb in pairs:
        d = _off0(sb) - _off0(sa)
        assert d % 2 == 0 and d >= 58, (sa, sb, d)
    return pairs


PAIRS_BY_HALF = [_make_pairs(lt) for lt in LO_TAPS]
NPAIRS = [len(p) for p in PAIRS_BY_HALF]  # [7, 6]



def _slot_off(slot, rloc):
    lvl, kh, kw = slot
    return (rloc + kh) * 2 * WP + lvl * WP + kw


# gating payload: img0-chunk0 x data + the first two half-0 weight pairs,
# packed per channel so ONE DMA with ONE packet per partition (<=2KB; DMA
# packet overhead dominates, not bytes) unblocks the first matmuls. The
# remaining half-0 pairs follow as the immediate next DMA and land before
# the (initially clock-throttled) stream reaches pair 2.
GPAIRS = 3                                 # weight pairs carried in the gate
GATE_X = CHUNKS[0][1] * 2 * WP             # 1160 elements of x rows 0:10
GATE_SZ = GATE_X + GPAIRS * 2 * P          # + 512 elements of weights
assert GATE_SZ <= 2048                     # one DMA packet per partition
WREST = NPAIRS[0] - GPAIRS                 # 5 half-0 pairs in the wb tensor


def build_nc():
    nc = bacc.Bacc(None, target_bir_lowering=False)
    x = nc.dram_tensor("x", [IMGS, C, HP, 2, WP], FP8, kind="ExternalInput")
    gate = nc.dram_tensor("gate", [C, GATE_SZ], FP8, kind="ExternalInput")
    wb = nc.dram_tensor("wb", [C, WREST + NPAIRS[1], 2, P], FP8,
                        kind="ExternalInput")
    out = nc.dram_tensor("out", [IMGS, O, H, WD], BF16, kind="ExternalOutput")

    with TileContext(nc) as tc:
        with (
            tc.tile_pool(name="wpool", bufs=1) as wpool,
            tc.tile_pool(name="xpool", bufs=1) as xpool,
            tc.tile_pool(name="opool", bufs=16) as opool,
            tc.tile_pool(name="psum", bufs=8, space="PSUM") as psum_pool,
        ):
            gt = wpool.tile([P, GATE_SZ], FP8, name="gt")
            wt = wpool.tile([P, WREST + NPAIRS[1], 2, P], FP8, name="wt")
            # warmup scratch: mostly uninitialized — the warmup matmuls only
            # exist to keep the PE busy (HAM ramp) during the input DMA and
            # their PSUM bank is never read. The 4-element memset is the
            # minimum write that lets the tile allocator place the tile.
            wsc = wpool.tile([P, 2, 448], FP8, name="wsc")
            # memset on the (otherwise idle) vector engine so the DMA-capable
            # engines start writing gating descriptors immediately
            nc.vector.memset(wsc[:, 0, 0:4], 0.0)

            xts = [
                xpool.tile([P, IMGS, nr, 2, WP], FP8, name=f"xc{ci}")
                for ci, (_, nr) in enumerate(CHUNKS)
            ]

            # DMA completion is dominated by per-packet overhead (~110ns,
            # one packet per partition -> 128/DMA regardless of bytes) and
            # the 16 HW engines serve all queues round-robin. To give the
            # gating transfers strict priority, ALL input DMAs go on sync's
            # single queue in need-order (per-queue FIFO): gating pieces
            # first, everything else behind them. Output drains use
            # scalar's queue so they never compete with inputs; gpsimd (the
            # cleanup engine whose exit drain scales with its queue depth)
            # only carries the two tiny final-block stores.
            # 16-packet engine-attach dummy (one packet per HW DMA engine);
            # never read — it only pulls all 16 engines onto the queue
            # before the gate transfer's packets arrive.
            dmy = wpool.tile([P, 4], FP8, name="dmy")
            nc.sync.dma_start(out=dmy[0:16], in_=gate[0:16, 0:4])
            nc.sync.dma_start(out=gt[:], in_=gate[:])
            nc.sync.dma_start(out=wt[:, 0:WREST], in_=wb[:, 0:WREST])
            nc.sync.dma_start(out=xts[1][:, 0],
                              in_=x[0, :, CHUNKS[1][0]:CHUNKS[1][0] + CHUNKS[1][1]])
            nc.sync.dma_start(out=wt[:, WREST:], in_=wb[:, WREST:])
            for ci, (r0, nr) in list(enumerate(CHUNKS))[2:]:
                nc.sync.dma_start(out=xts[ci][:, 0], in_=x[0, :, r0:r0 + nr])
            for img in range(1, IMGS):
                for ci, (r0, nr) in enumerate(CHUNKS):
                    nc.sync.dma_start(out=xts[ci][:, img], in_=x[img, :, r0:r0 + nr])

            warm = psum_pool.tile([P, RB, WD], F32, name="warm", tag="pst")
            for _ in range(N_WARM):
                nc.tensor.matmul(
                    warm[:], lhsT=wsc[:, :, :P], rhs=wsc[:],
                    start=True, stop=True, perf_mode=DR,
                )

            for img in range(IMGS):
                for half in range(2):
                    pairs = PAIRS_BY_HALF[half]
                    otd = None
                    for blk in range(NBLK):
                        ci = BLK_CHUNK[blk]
                        nr = CHUNKS[ci][1]
                        rloc = blk * RB - CHUNKS[ci][0]
                        if img == 0 and ci == 0:
                            base = gt[:]
                            ppitch = GATE_SZ
                            img_off = 0
                        else:
                            base = xts[ci][:]
                            ppitch = IMGS * nr * 2 * WP
                            img_off = img * nr * 2 * WP
                        last = (img == IMGS - 1 and half == 1
                                and blk == NBLK - 1)
                        # the final block runs as two 4-row PSUM groups so
                        # the first drain+store overlaps the second group's
                        # matmuls, shortening the kernel tail.
                        subs = (0, 1) if last else (0,)
                        rows = RB // 2 if last else RB
                        for sub in subs:
                            r0b = rloc + sub * rows
                            pst = psum_pool.tile([P, rows, WD], F32,
                                                 name="pst", tag="pst")
                            for pi, (sa, sb) in enumerate(pairs):
                                offa = _slot_off(sa, r0b)
                                offb = _slot_off(sb, r0b)
                                rhs = AP(
                                    tensor=base.tensor,
                                    offset=base.offset + img_off + offa,
                                    ap=[[ppitch, P], [offb - offa, 2],
                                        [2 * WP, rows], [1, WD]],
                                )
                                if half == 0 and pi < GPAIRS:
                                    lhsT = AP(
                                        tensor=gt.tensor,
                                        offset=gt.offset + GATE_X + pi * 2 * P,
                                        ap=[[GATE_SZ, P], [P, 2], [1, P]],
                                    )
                                elif half == 0:
                                    lhsT = wt[:, pi - GPAIRS]
                                else:
                                    lhsT = wt[:, WREST + pi]
                                nc.tensor.matmul(
                                    pst[:],
                                    lhsT=lhsT,
                                    rhs=rhs,
                                    start=(pi == 0),
                                    stop=(pi == len(pairs) - 1),
                                    perf_mode=DR,
                                )
                            orow = blk * RB + sub * rows
                            if last:
                                if sub == 0:
                                    ot = opool.tile([P, rows, WD], BF16,
                                                    name="ot", tag="ot")
                                    nc.vector.tensor_copy(ot[:], pst[:])
                                    nc.sync.dma_start(
                                        out=out[img, half * P:(half + 1) * P,
                                                orow:orow + rows, :],
                                        in_=ot[:],
                                    )
                                else:
                                    # final drain: independent ot tiles,
                                    # idle engines (gpsimd/vector copies,
                                    # gpsimd/sync stores) so the copies and
                                    # descriptor writes all run in parallel
                                    # off scalar's busy queue
                                    ota = opool.tile([P, rows // 2, WD],
                                                     BF16, name="ota",
                                                     tag="ota")
                                    otb = opool.tile([P, rows // 2, WD],
                                                     BF16, name="otb",
                                                     tag="otb")
                                    nc.scalar.copy(
                                        out=ota[:], in_=pst[:, 0:rows // 2])
                                    nc.gpsimd.dma_start(
                                        out=out[img, half * P:(half + 1) * P,
                                                orow:orow + rows // 2, :],
                                        in_=ota[:],
                                    )
                                    nc.vector.tensor_copy(
                                        otb[:], pst[:, rows // 2:])
                                    nc.sync.dma_start(
                                        out=out[img, half * P:(half + 1) * P,
                                                orow + rows // 2:orow + rows,
                                                :],
                                        in_=otb[:],
                                    )
                            else:
                                # all drain copies on vector (scalar's
                                # engine time goes to store descriptors);
                                # two consecutive blocks drain into one
                                # double-wide tile and ship as ONE store
                                # DMA — half the descriptor writes, and
                                # 1792B packets (vs 896B) cost the same
                                # per-packet overhead.
                                if blk % 2 == 0:
                                    otd = opool.tile([P, 2, RB, WD], BF16,
                                                     name="otd", tag="ot")
                                nc.vector.tensor_copy(otd[:, blk % 2],
                                                      pst[:])
                                if blk % 2 == 1:
                                    nc.scalar.dma_start(
                                        out=out[img,
                                                half * P:(half + 1) * P,
                                                (blk - 1) * RB:
                                                (blk + 1) * RB, :],
                                        in_=otd[:],
                                    )
                                elif blk == NBLK - 1:
                                    # odd block count: last block ships solo
                                    nc.scalar.dma_start(
                                        out=out[img,
                                                half * P:(half + 1) * P,
                                                orow:orow + rows, :],
                                        in_=otd[:, 0],
                                    )
    nc.compile()
    return nc


_NC_CACHE = None


def _get_nc():
    global _NC_CACHE
    if _NC_CACHE is None:
        _NC_CACHE = build_nc()
    return _NC_CACHE


def prep_inputs(x: np.ndarray, W: np.ndarray):
    xf = np.asarray(x, dtype=np.float32)
    x_hi = xf.astype(NP_FP8)
    x_lo = (xf - x_hi.astype(np.float32)).astype(NP_FP8)
    xp = np.zeros((xf.shape[0], C, HP, 2, WP), dtype=NP_FP8)
    xp[:, :, 1:H + 1, 0, 1:WD + 1] = x_hi
    xp[:, :, 1:H + 1, 1, 1:WD + 1] = x_lo
    wsign = np.sign(np.asarray(W, dtype=np.float32)).astype(NP_FP8)
    # [O,C,3,3] -> [C, half, kh, kw, 128]
    wbt = wsign.reshape(2, P, C, KH, KW).transpose(2, 0, 3, 4, 1)
    wq = [np.zeros((C, n, 2, P), dtype=NP_FP8) for n in NPAIRS]
    for half in range(2):
        for pi, (sa, sb) in enumerate(PAIRS_BY_HALF[half]):
            for g, slot in enumerate((sa, sb)):
                _, kh, kw = slot
                wq[half][:, pi, g, :] = wbt[:, half, kh, kw, :]
    xs = xp.reshape(N_CORES, IMGS, C, HP, 2, WP)
    wrest = np.concatenate([wq[0][:, GPAIRS:], wq[1]], axis=1)
    maps = []
    for c in range(N_CORES):
        gatec = np.empty((C, GATE_SZ), dtype=NP_FP8)
        gatec[:, :GATE_X] = xs[c, 0, :, 0:CHUNKS[0][1]].reshape(C, GATE_X)
        gatec[:, GATE_X:] = wq[0][:, :GPAIRS].reshape(C, GPAIRS * 2 * P)
        maps.append({
            "x": np.ascontiguousarray(xs[c]),
            "gate": gatec,
            "wb": wrest,
        })
    return maps


def kernel(x: np.ndarray, W: np.ndarray) -> np.ndarray:
    nc = _get_nc()
    in_maps = prep_inputs(x, W)
    res = run_bass_kernel_spmd(nc, in_maps, core_ids=list(range(N_CORES)))
    outs = [res.results[c]["out"] for c in range(N_CORES)]
    return np.concatenate(outs, axis=0).astype(np.float32)


# revision 51
# speedup vs baseline: 1.0010x; 1.0010x over previous
"""Binary-weight 3x3 conv (sign(W)), NCHW, stride 1, pad 1, on 8 trn2 cores.

Data-parallel over batch (4 images/core), implicit GEMM in fp8 DoubleRow.

Numerics: x is split host-side into x_hi = e4m3(x) and x_lo = e4m3(x - x_hi),
row-interleaved in one padded buffer. The contraction runs hi taps (all 9)
plus a PARTIAL residual (lo) correction, packed two-per-DoubleRow-matmul.
The correction is ASYMMETRIC across the two output-channel halves: half 0
corrects 5 taps (center + 4 edges -> 14 slots -> 7 DR matmuls), half 1
corrects 3 taps (center + 2 edges -> 12 slots -> 6 DR matmuls). Total
13 DR matmuls per (img, row-block) instead of 14: ~7% fewer PE cycles.
Correcting interior taps first is variance-optimal (border taps multiply
zero padding on 1/56 of outputs, so they carry slightly less error).

Error model (verified to 0.5% against HW): e4m3 quantization gives rel L2
0.02667; correcting a tap of weight w_t (center 1.0, edge 55/56, corner
(55/56)^2, sum 8.787) scales the variance by (8.787 - sum w_corr)/8.787
per half. Host-sim of this scheme = 0.01969; HW adds ~0.0017 in quadrature
(bf16 output store) -> ~0.01976 < 2e-2.

HW constraints honored: the DR ifmap group stride (dim-1 of the custom
4-dim access pattern) must be even and not tiny -- odd strides fault the
device -- so each parity class (by kw, since WP is even) must have an even
slot count; pairs are first-half x second-half within a class (deltas
116/174 elements, all even).

Schedule: DMA completion cost is per-packet (one packet per partition,
~110ns each across 16 HW engines that serve all queues round-robin), so
ALL input DMAs ride sync's single queue in need-order: a 16-packet dummy
that attaches every HW engine to the queue (the 16th otherwise joins
~1.9us late and its slice straggles), then one <=2KB/partition gating
packet carrying img0's first rows plus the first 3 weight pairs, then the
rest in stream order. Output stores ride scalar's queue so they never
queue ahead of inputs; gpsimd (whose exit drain scales with its queue
depth) carries only the final two tiny stores. Uninitialized-scratch
warmup matmuls bridge preamble-end to gating-data-ready with no PE idle
gap, so the HAM clock ramp (needs ~3.4us of continuous PE busy to reach
2.4 GHz) completes before the real stream starts. PSUM is drained to bf16
(halving store traffic, upcast on host) on alternating Vector/Scalar
engines; the final block is computed as two 4-row PSUM groups with
independent ot tiles across engines/queues so the last drain+store chain
is as short as possible.
"""

import numpy as np

import concourse.bacc as bacc
import concourse.mybir as mybir
from concourse.ap import AP
from concourse.tile import TileContext
from concourse.bass_utils import run_bass_kernel_spmd

N_CORES = 8
IMGS = 4
C = 128
O = 256
H = WD = 56
HP = WP = 58
KH = KW = 3
RB = 8
NBLK = H // RB
P = 128
# warmup matmuls bridge from preamble-end (~7.5us) to gating-data-ready
# with NO PE idle gap, so the HAM clock ramp (needs ~3.4us of continuous
# PE busy) completes before the stream. A 16-packet dummy DMA rides ahead
# of the gate so all 16 HW DMA engines attach to the queue early (the 16th
# engine otherwise attaches ~1.9us after the doorbell and its slice of the
# gate becomes a ~1.5us straggler).
N_WARM = 10

CHUNKS = [(0, 10), (8, 18), (24, 18), (40, 18)]
BLK_CHUNK = [0, 1, 1, 2, 2, 3, 3]

F32 = mybir.dt.float32
BF16 = mybir.dt.bfloat16
FP8 = mybir.dt.float8e4
NP_FP8 = mybir.dt.np(FP8)
DR = mybir.MatmulPerfMode.DoubleRow

# Residual (lo) taps corrected per output-channel half, interior-first
# (variance-optimal) while keeping each kw-parity class even-sized for
# DoubleRow pairing.
LO_TAPS = [
    [(1, 0), (1, 2), (0, 1), (1, 1), (2, 1)],  # half 0: center + 4 edges
    [(1, 0), (1, 2), (1, 1)],                  # half 1: center + 2 edges
]


# slot = (lvl, kh, kw); offset within an image-row block = kh*2*WP + lvl*WP + kw
def _off0(slot):
    lvl, kh, kw = slot
    return kh * 2 * WP + lvl * WP + kw


def _make_pairs(lo_taps):
    slots = [(0, kh, kw) for kh in range(KH) for kw in range(KW)]
    slots += [(1, kh, kw) for kh, kw in lo_taps]
    pairs = []
    for parity in (0, 1):
        cls = sorted((s for s in slots if _off0(s) % 2 == parity), key=_off0)
        assert len(cls) % 2 == 0, (parity, cls)
        h = len(cls) // 2
        pairs += list(zip(cls[:h], cls[h:]))
    for sa, sb in pairs:
        d = _off0(sb) - _off0(sa)
        assert d % 2 == 0 and d >= 58, (sa, sb, d)
    return pairs


PAIRS_BY_HALF = [_make_pairs(lt) for lt in LO_TAPS]
NPAIRS = [len(p) for p in PAIRS_BY_HALF]  # [7, 6]



def _slot_off(slot, rloc):
    lvl, kh, kw = slot
    return (rloc + kh) * 2 * WP + lvl * WP + kw


# gating payload: img0-chunk0 x data + the first two half-0 weight pairs,
# packed per channel so ONE DMA with ONE packet per partition (<=2KB; DMA
# packet overhead dominates, not bytes) unblocks the first matmuls. The
# remaining half-0 pairs follow as the immediate next DMA and land before
# the (initially clock-throttled) stream reaches pair 2.
GPAIRS = 3                                 # weight pairs carried in the gate
GATE_X = CHUNKS[0][1] * 2 * WP             # 1160 elements of x rows 0:10
GATE_SZ = GATE_X + GPAIRS * 2 * P          # + 512 elements of weights
assert GATE_SZ <= 2048                     # one DMA packet per partition
WREST = NPAIRS[0] - GPAIRS                 # 5 half-0 pairs in the wb tensor


def build_nc():
    nc = bacc.Bacc(None, target_bir_lowering=False)
    x = nc.dram_tensor("x", [IMGS, C, HP, 2, WP], FP8, kind="ExternalInput")
    gate = nc.dram_tensor("gate", [C, GATE_SZ], FP8, kind="ExternalInput")
    wb = nc.dram_tensor("wb", [C, WREST + NPAIRS[1], 2, P], FP8,
                        kind="ExternalInput")
    out = nc.dram_tensor("out", [IMGS, O, H, WD], BF16, kind="ExternalOutput")

    with TileContext(nc) as tc:
        with (
            tc.tile_pool(name="wpool", bufs=1) as wpool,
            tc.tile_pool(name="xpool", bufs=1) as xpool,
            tc.tile_pool(name="opool", bufs=16) as opool,
            tc.tile_pool(name="psum", bufs=8, space="PSUM") as psum_pool,
        ):
            gt = wpool.tile([P, GATE_SZ], FP8, name="gt")
            wt = wpool.tile([P, WREST + NPAIRS[1], 2, P], FP8, name="wt")
            # warmup scratch: mostly uninitialized — the warmup matmuls only
            # exist to keep the PE busy (HAM ramp) during the input DMA and
            # their PSUM bank is never read. The 4-element memset is the
            # minimum write that lets the tile allocator place the tile.
            wsc = wpool.tile([P, 2, 448], FP8, name="wsc")
            # memset on the (otherwise idle) vector engine so the DMA-capable
            # engines start writing gating descriptors immediately
            nc.vector.memset(wsc[:, 0, 0:4], 0.0)

            # img0 arrives as row-chunks (its blocks have tight deadlines
            # right after stream start); imgs 1-3 arrive as ONE whole-image
            # DMA each — 128 packets of 6728B instead of 512 of ~1-2KB
            # (per-packet overhead dominates) and one descriptor write
            # instead of four.
            xts = [
                xpool.tile([P, nr, 2, WP], FP8, name=f"xc{ci}")
                for ci, (_, nr) in enumerate(CHUNKS)
            ]
            xfull = xpool.tile([P, IMGS - 1, HP, 2, WP], FP8, name="xfull")

            # DMA completion is dominated by per-packet overhead (~110ns,
            # one packet per partition -> 128/DMA regardless of bytes) and
            # the 16 HW engines serve all queues round-robin. To give the
            # gating transfers strict priority, ALL input DMAs go on sync's
            # single queue in need-order (per-queue FIFO): gating pieces
            # first, everything else behind them. Output drains use
            # scalar's queue so they never compete with inputs; gpsimd (the
            # cleanup engine whose exit drain scales with its queue depth)
            # only carries the two tiny final-block stores.
            # 16-packet engine-attach dummy (one packet per HW DMA engine);
            # never read — it only pulls all 16 engines onto the queue
            # before the gate transfer's packets arrive.
            dmy = wpool.tile([P, 4], FP8, name="dmy")
            nc.sync.dma_start(out=dmy[0:16], in_=gate[0:16, 0:4])
            nc.sync.dma_start(out=gt[:], in_=gate[:])
            nc.sync.dma_start(out=wt[:, 0:WREST], in_=wb[:, 0:WREST])
            nc.sync.dma_start(out=xts[1][:],
                              in_=x[0, :, CHUNKS[1][0]:CHUNKS[1][0] + CHUNKS[1][1]])
            nc.sync.dma_start(out=wt[:, WREST:], in_=wb[:, WREST:])
            for ci, (r0, nr) in list(enumerate(CHUNKS))[2:]:
                nc.sync.dma_start(out=xts[ci][:], in_=x[0, :, r0:r0 + nr])
            for img in range(1, IMGS):
                nc.sync.dma_start(out=xfull[:, img - 1], in_=x[img])

            warm = psum_pool.tile([P, RB, WD], F32, name="warm", tag="pst")
            for _ in range(N_WARM):
                nc.tensor.matmul(
                    warm[:], lhsT=wsc[:, :, :P], rhs=wsc[:],
                    start=True, stop=True, perf_mode=DR,
                )

            for img in range(IMGS):
                for half in range(2):
                    pairs = PAIRS_BY_HALF[half]
                    otd = None
                    for blk in range(NBLK):
                        ci = BLK_CHUNK[blk]
                        nr = CHUNKS[ci][1]
                        rloc = blk * RB - CHUNKS[ci][0]
                        if img == 0 and ci == 0:
                            base = gt[:]
                            ppitch = GATE_SZ
                            img_off = 0
                        elif img == 0:
                            base = xts[ci][:]
                            ppitch = nr * 2 * WP
                            img_off = 0
                        else:
                            base = xfull[:]
                            ppitch = (IMGS - 1) * HP * 2 * WP
                            img_off = (img - 1) * HP * 2 * WP
                            rloc = blk * RB
                        last = (img == IMGS - 1 and half == 1
                                and blk == NBLK - 1)
                        # the final block runs as two 4-row PSUM groups so
                        # the first drain+store overlaps the second group's
                        # matmuls, shortening the kernel tail.
                        subs = (0, 1) if last else (0,)
                        rows = RB // 2 if last else RB
                        for sub in subs:
                            r0b = rloc + sub * rows
                            pst = psum_pool.tile([P, rows, WD], F32,
                                                 name="pst", tag="pst")
                            for pi, (sa, sb) in enumerate(pairs):
                                offa = _slot_off(sa, r0b)
                                offb = _slot_off(sb, r0b)
                                rhs = AP(
                                    tensor=base.tensor,
                                    offset=base.offset + img_off + offa,
                                    ap=[[ppitch, P], [offb - offa, 2],
                                        [2 * WP, rows], [1, WD]],
                                )
                                if half == 0 and pi < GPAIRS:
                                    lhsT = AP(
                                        tensor=gt.tensor,
                                        offset=gt.offset + GATE_X + pi * 2 * P,
                                        ap=[[GATE_SZ, P], [P, 2], [1, P]],
                                    )
                                elif half == 0:
                                    lhsT = wt[:, pi - GPAIRS]
                                else:
                                    lhsT = wt[:, WREST + pi]
                                nc.tensor.matmul(
                                    pst[:],
                                    lhsT=lhsT,
                                    rhs=rhs,
                                    start=(pi == 0),
                                    stop=(pi == len(pairs) - 1),
                                    perf_mode=DR,
                                )
                            orow = blk * RB + sub * rows
                            if last:
                                if sub == 0:
                                    ot = opool.tile([P, rows, WD], BF16,
                                                    name="ot", tag="ot")
                                    nc.vector.tensor_copy(ot[:], pst[:])
                                    nc.sync.dma_start(
                                        out=out[img, half * P:(half + 1) * P,
                                                orow:orow + rows, :],
                                        in_=ot[:],
                                    )
                                else:
                                    # final drain: independent ot tiles,
                                    # idle engines (gpsimd/vector copies,
                                    # gpsimd/sync stores) so the copies and
                                    # descriptor writes all run in parallel
                                    # off scalar's busy queue
                                    ota = opool.tile([P, rows // 2, WD],
                                                     BF16, name="ota",
                                                     tag="ota")
                                    otb = opool.tile([P, rows // 2, WD],
                                                     BF16, name="otb",
                                                     tag="otb")
                                    nc.scalar.copy(
                                        out=ota[:], in_=pst[:, 0:rows // 2])
                                    nc.gpsimd.dma_start(
                                        out=out[img, half * P:(half + 1) * P,
                                                orow:orow + rows // 2, :],
                                        in_=ota[:],
                                    )
                                    nc.vector.tensor_copy(
                                        otb[:], pst[:, rows // 2:])
                                    nc.sync.dma_start(
                                        out=out[img, half * P:(half + 1) * P,
                                                orow + rows // 2:orow + rows,
                                                :],
                                        in_=otb[:],
                                    )
                            else:
                                # all drain copies on vector (scalar's
                                # engine time goes to store descriptors);
                                # two consecutive blocks drain into one
                                # double-wide tile and ship as ONE store
                                # DMA — half the descriptor writes, and
                                # 1792B packets (vs 896B) cost the same
                                # per-packet overhead.
                                if blk % 2 == 0:
                                    otd = opool.tile([P, 2, RB, WD], BF16,
                                                     name="otd", tag="ot")
                                nc.vector.tensor_copy(otd[:, blk % 2],
                                                      pst[:])
                                if blk % 2 == 1:
                                    nc.scalar.dma_start(
                                        out=out[img,
                                                half * P:(half + 1) * P,
                                                (blk - 1) * RB:
                                                (blk + 1) * RB, :],
                                        in_=otd[:],
                                    )
                                elif blk == NBLK - 1:
                                    # odd block count: last block ships solo
                                    nc.scalar.dma_start(
                                        out=out[img,
                                                half * P:(half + 1) * P,
                                                orow:orow + rows, :],
                                        in_=otd[:, 0],
                                    )
    nc.compile()
    return nc


_NC_CACHE = None


def _get_nc():
    global _NC_CACHE
    if _NC_CACHE is None:
        _NC_CACHE = build_nc()
    return _NC_CACHE


def prep_inputs(x: np.ndarray, W: np.ndarray):
    xf = np.asarray(x, dtype=np.float32)
    x_hi = xf.astype(NP_FP8)
    x_lo = (xf - x_hi.astype(np.float32)).astype(NP_FP8)
    xp = np.zeros((xf.shape[0], C, HP, 2, WP), dtype=NP_FP8)
    xp[:, :, 1:H + 1, 0, 1:WD + 1] = x_hi
    xp[:, :, 1:H + 1, 1, 1:WD + 1] = x_lo
    wsign = np.sign(np.asarray(W, dtype=np.float32)).astype(NP_FP8)
    # [O,C,3,3] -> [C, half, kh, kw, 128]
    wbt = wsign.reshape(2, P, C, KH, KW).transpose(2, 0, 3, 4, 1)
    wq = [np.zeros((C, n, 2, P), dtype=NP_FP8) for n in NPAIRS]
    for half in range(2):
        for pi, (sa, sb) in enumerate(PAIRS_BY_HALF[half]):
            for g, slot in enumerate((sa, sb)):
                _, kh, kw = slot
                wq[half][:, pi, g, :] = wbt[:, half, kh, kw, :]
    xs = xp.reshape(N_CORES, IMGS, C, HP, 2, WP)
    wrest = np.concatenate([wq[0][:, GPAIRS:], wq[1]], axis=1)
    maps = []
    for c in range(N_CORES):
        gatec = np.empty((C, GATE_SZ), dtype=NP_FP8)
        gatec[:, :GATE_X] = xs[c, 0, :, 0:CHUNKS[0][1]].reshape(C, GATE_X)
        gatec[:, GATE_X:] = wq[0][:, :GPAIRS].reshape(C, GPAIRS * 2 * P)
        maps.append({
            "x": np.ascontiguousarray(xs[c]),
            "gate": gatec,
            "wb": wrest,
        })
    return maps


def kernel(x: np.ndarray, W: np.ndarray) -> np.ndarray:
    nc = _get_nc()
    in_maps = prep_inputs(x, W)
    res = run_bass_kernel_spmd(nc, in_maps, core_ids=list(range(N_CORES)))
    outs = [res.results[c]["out"] for c in range(N_CORES)]
    return np.concatenate(outs, axis=0).astype(np.float32)
